# revision 16
# baseline (speedup 1.0000x reference)
"""Trainium2 Bass kernel for nn_AffinityMah (retrieval_knn).

Math (per batch b):
    out[n, m] = relu( ||Y[b,n] @ A||^2 + ||X[b,m] @ A||^2 - 2 * (YA @ XA^T)[n, m] )

Strategy:
  - Data-parallel over batch B=8 across the 8 NeuronCores (one batch per core).
  - Inputs are cast to bf16 on the host (halves input HBM traffic; the PE runs
    bf16 matmuls at 1 cycle/row with fast weight load).
  - X^T / Y^T are produced with PE transposes of 128x128 bf16 tiles (the DMA
    crossbar transpose hangs on this runtime), then DVE copies PSUM -> SBUF.
  - XA^T / YA^T slices come from matmuls against A chunks (contract D=256 in
    two 128-chunks, accumulate in PSUM); row-sums of squares from a
    ones-vector matmul over Square(XA^T).
  - The whole quadratic form is then ONE TensorE matmul per (128, 512) output
    tile via an augmented contraction dim K+2 = 102:
        lhsT rows 0..99  = YA^T            rhs rows 0..99  = -2 * XA^T
        lhsT row  100    = sqY             rhs row  100    = ones
        lhsT row  101    = ones            rhs row  101    = sqX
    giving out_tile = sqY[:,None] + sqX[None,:] - 2*cross directly in PSUM.
    A relu copy (ACT/DVE alternating) moves each tile to SBUF and a 256 KB
    DMA writes it out immediately (wavefront order so output DMA starts
    as early as possible).
"""

import numpy as np

B, MX, NY, D, K = 8, 2048, 2048, 256, 100
KP = K + 2  # augmented contraction dim
S = 512     # moving-operand slice width
NS = MX // S          # 4 column slices
JT = NY // 128        # 16 output row blocks

_NC = None


def _emit(tc, O, X, Y, A, ID):
    from contextlib import ExitStack

    import concourse.mybir as mybir

    nc = tc.nc
    f32 = mybir.dt.float32
    bf16 = mybir.dt.bfloat16
    AF = mybir.ActivationFunctionType

    with ExitStack() as ctx:
        const = ctx.enter_context(tc.tile_pool(name="const", bufs=1))
        lr = ctx.enter_context(tc.tile_pool(name="lr", bufs=1))
        xin = ctx.enter_context(tc.tile_pool(name="xin", bufs=3))
        xt = ctx.enter_context(tc.tile_pool(name="xt", bufs=3))
        sqp = ctx.enter_context(tc.tile_pool(name="sqp", bufs=2))
        obp = ctx.enter_context(tc.tile_pool(name="obp", bufs=6))
        pt = ctx.enter_context(tc.tile_pool(name="pt", bufs=2, space="PSUM"))
        pa = ctx.enter_context(tc.tile_pool(name="pa", bufs=1, space="PSUM"))
        ps = ctx.enter_context(tc.tile_pool(name="ps", bufs=1, space="PSUM"))
        po = ctx.enter_context(tc.tile_pool(name="po", bufs=4, space="PSUM"))

        # identity shipped as a DRAM constant input: a gpsimd-built identity
        # (memset + affine_select) delays the first PE transpose by several us
        ident = const.tile([128, 128], bf16, name="ident")
        nc.sync.dma_start(ident[:], ID[:])

        a_chunks = []
        for c in range(2):
            ac = const.tile([128, K], bf16, name=f"a{c}", tag=f"a{c}")
            nc.sync.dma_start(ac[:], A[c * 128:(c + 1) * 128, :])
            a_chunks.append(ac)

        ones_w = const.tile([K, 1], bf16, name="ones_w", tag="ones_w")
        nc.vector.memset(ones_w[:], 1.0)
        ones_row = const.tile([1, S], bf16, name="ones_row", tag="ones_row")
        nc.vector.memset(ones_row[:], 1.0)

        # L parts: [YA^T; sqY; ones], R parts: [-2 XA^T; ones; sqX]
        Lp, Rp = [], []
        for s in range(NS):
            lt = lr.tile([KP, S], bf16, name=f"L{s}", tag=f"L{s}")
            Lp.append(lt)
            rt = lr.tile([KP, S], bf16, name=f"R{s}", tag=f"R{s}")
            Rp.append(rt)

        # ---- Stage A + main loop, interleaved by wavefront ----
        # All slab loads first so transposes unblock as fast as DMA allows.
        slabs = {}
        for s in range(NS):
            for ti, T in ((1, Y), (0, X)):
                slab = xin.tile([128, NS, D], bf16, name=f"slab{ti}{s}",
                                tag=f"slab{ti}{s}")
                nc.gpsimd.dma_start(
                    slab[:],
                    T[s * S:(s + 1) * S, :].rearrange("(u p) d -> p u d", p=128),
                )
                slabs[ti, s] = slab

        for s in range(NS):
            for ti, T in ((1, Y), (0, X)):
                slab = slabs[ti, s]
                xts = [
                    xt.tile([128, S], bf16, name=f"xt{ti}{s}{c}", tag=f"xt{c}")
                    for c in range(2)
                ]
                for u in range(S // 128):
                    for c in range(2):
                        ptile = pt.tile([128, 128], bf16,
                                        name=f"pt{ti}{s}{u}{c}", tag="pt")
                        nc.tensor.transpose(
                            ptile[:], slab[:, u, c * 128:(c + 1) * 128], ident[:]
                        )
                        nc.vector.tensor_copy(
                            xts[c][:, u * 128:(u + 1) * 128], ptile[:]
                        )

                # XA^T / YA^T slice: accumulate over the two D-chunks
                pxa = pa.tile([K, S], f32, name=f"pxa{ti}{s}", tag="pa")
                nc.tensor.matmul(pxa[:], a_chunks[0][:], xts[0][:],
                                 start=True, stop=False)
                nc.tensor.matmul(pxa[:], a_chunks[1][:], xts[1][:],
                                 start=False, stop=True)

                sqt = sqp.tile([K, S], bf16, name=f"sq{ti}{s}", tag="sq")
                nc.scalar.square(sqt[:], pxa[:])
                if ti == 0:
                    nc.scalar.mul(Rp[s][0:K, :], pxa[:], -2.0)
                else:
                    nc.scalar.copy(Lp[s][0:K, :], pxa[:])

                pss = ps.tile([1, S], f32, name=f"pss{ti}{s}", tag="ps")
                nc.tensor.matmul(pss[:], ones_w[:], sqt[:], start=True, stop=True)

                # rows 100 (L: sqY / R: ones) and 101 (L: ones / R: sqX):
                # compute writes must start 32-aligned, so stage the sq row at
                # partition 0 and DMA rows into place individually.
                sqrow = sqp.tile([1, S], bf16, name=f"sqrow{ti}{s}", tag="sqrow")
                nc.vector.tensor_copy(sqrow[:], pss[:])
                if ti == 0:
                    nc.sync.dma_start(Rp[s][K:K + 1, :], ones_row[:])
                    nc.sync.dma_start(Rp[s][K + 1:K + 2, :], sqrow[:])
                else:
                    nc.sync.dma_start(Lp[s][K:K + 1, :], sqrow[:])
                    nc.sync.dma_start(Lp[s][K + 1:K + 2, :], ones_row[:])

        # ---- Main loop: paired-t tiles, wave order (earliest-ready first) ----
        # pair th covers t in {2*th, 2*th+1}; ready once slices up to
        # max(j//4, 2*th+1) are built
        pairs = [(j, th) for j in range(JT) for th in range(NS // 2)]
        pairs.sort(key=lambda p: (max(p[0] // 4, 2 * p[1] + 1), p[1], p[0]))
        relu_i = 0
        for j, th in pairs:
            ot = obp.tile([128, 2 * S], f32, name=f"ot{j}_{th}", tag="ot")
            for k in range(2):
                t = 2 * th + k
                pot = po.tile([128, S], f32, name=f"po{j}_{t}", tag="po")
                nc.tensor.matmul(
                    pot[:],
                    Lp[j // 4][:, (j % 4) * 128:(j % 4 + 1) * 128],
                    Rp[t][:],
                    start=True, stop=True,
                )
                if relu_i % 2 == 0:
                    nc.scalar.activation(ot[:, k * S:(k + 1) * S], pot[:], AF.Relu)
                else:
                    nc.vector.tensor_relu(ot[:, k * S:(k + 1) * S], pot[:])
                relu_i += 1
            nc.sync.dma_start(
                O[j * 128:(j + 1) * 128, 2 * th * S:(2 * th + 2) * S], ot[:]
            )


def _build_nc():
    import concourse.bass as bass  # noqa: F401
    import concourse.mybir as mybir
    import concourse.tile as tile
    from concourse import bacc

    f32 = mybir.dt.float32
    bf16 = mybir.dt.bfloat16
    nc = bacc.Bacc(
        "TRN2", target_bir_lowering=False, debug=False, enable_asserts=False
    )
    Xd = nc.dram_tensor("X", [MX, D], bf16, kind="ExternalInput").ap()
    Yd = nc.dram_tensor("Y", [NY, D], bf16, kind="ExternalInput").ap()
    Ad = nc.dram_tensor("A", [D, K], bf16, kind="ExternalInput").ap()
    IDd = nc.dram_tensor("IDENT", [128, 128], bf16, kind="ExternalInput").ap()
    Od = nc.dram_tensor("O", [NY, MX], f32, kind="ExternalOutput").ap()

    with tile.TileContext(nc) as tc:
        _emit(tc, Od, Xd, Yd, Ad, IDd)
    nc.compile()
    return nc


def get_nc():
    global _NC
    if _NC is None:
        _NC = _build_nc()
    return _NC


def kernel(X, Y, A, _trace=False):
    import ml_dtypes

    from concourse.bass_utils import run_bass_kernel_spmd

    nc = get_nc()
    bf16 = ml_dtypes.bfloat16
    Xb = np.ascontiguousarray(X, dtype=np.float32).astype(bf16)
    Yb = np.ascontiguousarray(Y, dtype=np.float32).astype(bf16)
    Ab = np.ascontiguousarray(A, dtype=np.float32).astype(bf16)
    ident = np.eye(128, dtype=bf16)
    in_maps = [{"X": Xb[b], "Y": Yb[b], "A": Ab, "IDENT": ident} for b in range(B)]
    res = run_bass_kernel_spmd(nc, in_maps, core_ids=list(range(B)), trace=_trace)
    out = np.stack([res.results[b]["O"] for b in range(B)], axis=0)
    if _trace:
        return out, res
    return out
